# revision 1
# baseline (speedup 1.0000x reference)
"""FFM layer kernel for 8 Trainium2 NeuronCores.

Math (reference): x[B,39] = 13 dense cols + 26 sparse index cols (ints 0..99
stored as f32).  inputs[B,2613] = [dense | one_hot(sparse)], then
  linear = inputs @ w.T + b
  field  = einsum('bn,nfk->bfk', inputs, v)        # [B,39,16]
  cross  = 0.5*sum_k((sum_f field)^2 - sum_f field^2)
  out    = sigmoid(linear + cross)

Strategy: data-parallel over batch, 2048 rows/core.  On each core the one-hot
matrix is built on-device (DVE is_equal against an offset ramp), transposed
feature-major so it can be the stationary matmul operand:
  psum[128b, 625] = sum_chunks ohT_chunk[128f,128b].T @ vperm_chunk[128f,625]
with 625 output cols = 624 field cols (k-major, f-minor) + 1 linear col.
Feature rows are [1s row (bias) | 13 dense | zero pad to 32 | 26*100
one-hot | zero tail], packed into 21 chunks of 128 so the PE contracts at
full K=128.  fp16 operands, fp32 PSUM accumulation.  Epilogue: strided
row-reduce for s[b,k], squared row-reduce for sum field^2, sigmoid on the
scalar engine.  Throwaway warmup matmuls release the HAM clock throttle
during the DMA head; host tensors are partition-major so DMAs move long
contiguous runs.
"""

import sys

sys.path.insert(0, "/opt/trn_rl_repo")

import numpy as np

import concourse.tile as tile
from concourse import bacc, mybir
from concourse.bass_utils import run_bass_kernel_spmd

N_CORES = 8
B_FULL = 16384
BC = B_FULL // N_CORES  # 2048 rows per core
P = 128
N_DENSE = 13
N_SPARSE = 26
SPARSE_DIM = 100
N_FIELD = 39
K_DIM = 16
NCHUNK = 21
RTOT = NCHUNK * P       # 2688 padded feature rows
# device feature rows: 0 = const-ones (bias), 1..13 = dense, 14..31 = zero
# pad (so the one-hot region starts 32-aligned for the compare ops), then
# 26*100 one-hot rows, then zero tail
SP0 = 32                # first one-hot row
NFEAT_END = SP0 + N_SPARSE * SPARSE_DIM  # 2632
COLS = N_FIELD * K_DIM + 1  # 625: 624 field cols + linear col
GB = 4                  # batch tiles per group (4 psum tiles = 8 banks)

F16 = mybir.dt.float16
F32 = mybir.dt.float32
I8 = mybir.dt.int8

_prog_cache = {}


def _build_program(bc):
    """One SPMD program for a batch slice of `bc` rows (all cores identical)."""
    nbt = bc // P
    ngroups = nbt // GB
    assert nbt % GB == 0
    gw = GB * P  # one-hot column width built per group

    nc = bacc.Bacc("TRN2", target_bir_lowering=False, debug=False)
    # idxrep/vperm are laid out partition-major on the host so every DMA
    # moves one long contiguous run per partition (short descriptor runs
    # throttle the DMA engines well below their 2KB+ peak efficiency)
    idx_d = nc.declare_dram_parameter(
        "idxrep", [ngroups, P, NCHUNK, gw], I8, isOutput=False)
    xdn_d = nc.declare_dram_parameter("xdn", [SP0, bc], F16, isOutput=False)
    vp_d = nc.declare_dram_parameter(
        "vperm", [P, NCHUNK, COLS], F16, isOutput=False)
    ramp_d = nc.declare_dram_parameter("ramp", [P, NCHUNK], F32, isOutput=False)
    # y kept [128, nbt] (partition-major) so the single output DMA writes
    # contiguous 64B runs instead of 2048 scattered 4B elements; the host
    # transposes the 8KB at the end
    y_d = nc.declare_dram_parameter("y", [P, nbt], F32, isOutput=True)

    # idx chunk sub-batches: one DMA each, alternating HWDGE queues so
    # descriptor generation overlaps; tiny first sub so chunk 0 lands early
    # and the first matmuls can start
    ISUB = [(0, 2), (2, 7), (7, 11), (11, 16), (16, NCHUNK)]
    ISUB_ENG = ("sync", "scalar", "sync", "scalar", "sync")
    VSUB = [(0, 2), (2, 9), (9, NCHUNK)]
    VSUB_ENG = ("scalar", "sync", "scalar")

    with tile.TileContext(nc) as tc:
        with (
            tc.tile_pool(name="pers", bufs=1) as pers,
            tc.tile_pool(name="idxp", bufs=2) as idxp,
            tc.tile_pool(name="psum", bufs=4, space="PSUM") as psum,
            tc.tile_pool(name="epi", bufs=3) as epi,
        ):
            oh_t = []
            for c in range(NCHUNK):
                oh_t.append(pers.tile([P, bc], F16, tag=f"oh{c}", name=f"oh{c}"))
            y_all = pers.tile([P, nbt], F32, tag="yall")
            vp_all = pers.tile([P, NCHUNK, COLS], F16, tag="vp")

            def load_idx(g):
                c0, c1 = g * gw, (g + 1) * gw
                subs = []
                for (lo, hi), ename in zip(ISUB, ISUB_ENG):
                    eng = getattr(nc, ename)
                    it = idxp.tile([P, hi - lo, gw], I8, tag=f"idx{lo}",
                                   name="idx", bufs=2)
                    eng.dma_start(it[:], idx_d[g, :, lo:hi, :])
                    subs.append((lo, it))
                return subs

            # ramp and the staged chunk-0 head rows first (tiny, and they
            # gate the first compare/copy), then group 0 idx loads
            ramp_t = pers.tile([P, NCHUNK], F32, tag="ramp")
            nc.sync.dma_start(ramp_t[:], ramp_d[:])

            def load_vp(lo, hi, ename):
                getattr(nc, ename).dma_start(
                    vp_all[:, lo:hi, :], vp_d[:, lo:hi, :])

            # vp chunks 0-1 first on scalar: they gate the first matmuls
            load_vp(*VSUB[0], VSUB_ENG[0])
            # rows 0..31 of chunk 0: row 0 = bias ones, 1..13 = dense x,
            # 14..31 = zeros (prebaked host-side); per group a single DVE
            # copy drops them into oh chunk 0.  The one-hot region starts at
            # row 32 so this never conflicts with the compares.
            xdn_t = pers.tile([SP0, bc], F16, tag="xdn")
            nc.scalar.dma_start(xdn_t[:], xdn_d[:])
            subs0 = load_idx(0)
            for (lo, hi), ename in list(zip(VSUB, VSUB_ENG))[1:]:
                load_vp(lo, hi, ename)

            # PE warmup: throwaway matmuls on zeroed tiles during the DMA
            # head release the HAM clock throttle (cold PE runs at 1.2GHz
            # until ~3.4us of sustained activity) so the real matmuls start
            # at 2.4GHz; the N=64 tail keeps the PE busy right up to when
            # the first one-hot chunk is ready without delaying it much
            wz16 = pers.tile([P, 16], F16, tag="wz16")
            wz512 = pers.tile([P, 512], F16, tag="wz512")
            nc.gpsimd.memset(wz16[:], 0.0)
            nc.gpsimd.memset(wz512[:], 0.0)
            wps = psum.tile([P, COLS], F32, tag="ps", name="warmps")
            for _ in range(10):
                nc.tensor.matmul(wps[0:16, 0:512], wz16[:], wz512[:],
                                 start=True, stop=True)
            for _ in range(40):
                nc.tensor.matmul(wps[0:16, 0:64], wz16[:], wz512[:, 0:64],
                                 start=True, stop=True)

            for g in range(ngroups):
                c0, c1 = g * gw, (g + 1) * gw
                subs = subs0 if g == 0 else load_idx(g)
                # one is_equal per chunk builds the one-hot columns; chunk 0
                # splits into [32:64)+[64:128) (start partitions must be
                # 32-aligned and 32-start allows at most 32 rows).  Group 0
                # builds in two column passes: a narrow bt0-only pass that
                # outruns PE consumption (~240ns vs 280ns per chunk), then
                # the rest while the PE works through bt0.
                passes = ((0, P), (P, gw)) if g == 0 else ((0, gw),)
                for pj0, pj1 in passes:
                    for si, (lo, it) in enumerate(subs):
                        for ci in range(it.shape[1]):
                            c = lo + ci
                            rngs = ((32, 64), (64, P)) if c == 0 else ((0, P),)
                            for r0, r1 in rngs:
                                nc.vector.tensor_scalar(
                                    out=oh_t[c][r0:r1, c0 + pj0:c0 + pj1],
                                    in0=it[r0:r1, ci, pj0:pj1],
                                    scalar1=ramp_t[r0:r1, c:c + 1],
                                    scalar2=None,
                                    op0=mybir.AluOpType.is_equal,
                                )
                        if si == 0:
                            # bias/dense/pad rows of chunk 0; issued after the
                            # first compares so a late xdn DMA can't
                            # head-of-line block the DVE queue
                            nc.vector.tensor_copy(
                                oh_t[0][0:SP0, c0 + pj0:c0 + pj1],
                                xdn_t[:, c0 + pj0:c0 + pj1])
                for b4 in range(GB):
                    bt = g * GB + b4
                    ps = psum.tile([P, COLS], F32, tag="ps")
                    for c in range(NCHUNK):
                        lhs = oh_t[c][:, bt * P:(bt + 1) * P]
                        nc.tensor.matmul(
                            ps[:, 0:512], lhs, vp_all[:, c, 0:512],
                            start=(c == 0), stop=(c == NCHUNK - 1),
                        )
                        nc.tensor.matmul(
                            ps[:, 512:COLS], lhs, vp_all[:, c, 512:COLS],
                            start=(c == 0), stop=(c == NCHUNK - 1),
                        )
                    # epilogue: s[b,k] = sum_f field, then cross + sigmoid
                    lin_t = epi.tile([P, 1], F32, tag="lin")
                    nc.vector.tensor_copy(lin_t[:], ps[:, COLS - 1:COLS])
                    s_t = epi.tile([P, K_DIM], F32, tag="s")
                    nc.vector.tensor_reduce(
                        out=s_t[:],
                        in_=ps[:, 0:COLS - 1].rearrange("p (k f) -> p k f", f=N_FIELD),
                        axis=mybir.AxisListType.X,
                        op=mybir.AluOpType.add,
                    )
                    sq_scr = epi.tile([P, COLS - 1], F32, tag="sqscr")
                    sqsum = epi.tile([P, 1], F32, tag="sqsum")
                    nc.scalar.activation(
                        out=sq_scr[:], in_=ps[:, 0:COLS - 1],
                        func=mybir.ActivationFunctionType.Square,
                        accum_out=sqsum[:],
                    )
                    # b2 = lin - 0.5*sqsum off the critical path: the final
                    # chain is then s_red -> square-accum -> sigmoid only
                    b2_t = epi.tile([P, 1], F32, tag="b2")
                    nc.vector.tensor_scalar(
                        out=b2_t[:], in0=sqsum[:],
                        scalar1=-0.5, scalar2=lin_t[:],
                        op0=mybir.AluOpType.mult,
                        op1=mybir.AluOpType.add,
                    )
                    s2_scr = epi.tile([P, K_DIM], F32, tag="s2scr")
                    s2sum = epi.tile([P, 1], F32, tag="s2sum")
                    nc.scalar.activation(
                        out=s2_scr[:], in_=s_t[:],
                        func=mybir.ActivationFunctionType.Square,
                        accum_out=s2sum[:],
                    )
                    nc.scalar.activation(
                        out=y_all[:, bt:bt + 1], in_=s2sum[:],
                        func=mybir.ActivationFunctionType.Sigmoid,
                        scale=0.5, bias=b2_t[:],
                    )
            nc.sync.dma_start(y_d[:], y_all[:])

    nc.compile()
    return nc


def _get_program(bc):
    if bc not in _prog_cache:
        _prog_cache[bc] = _build_program(bc)
    return _prog_cache[bc]


def _prep_shared(w_weight, w_bias, v):
    """vperm[RTOT, 625] fp16 and ramp[128, 21] f32 (same on every core)."""
    vperm = np.zeros((RTOT, COLS), np.float32)
    # cols j<624: j = k*39 + f  <->  v[n, f, k];  col 624 = linear weight
    v2 = np.ascontiguousarray(v.transpose(0, 2, 1)).reshape(2613, COLS - 1)
    vperm[1:1 + N_DENSE, :COLS - 1] = v2[:N_DENSE]
    vperm[1:1 + N_DENSE, COLS - 1] = w_weight[0, :N_DENSE]
    vperm[SP0:NFEAT_END, :COLS - 1] = v2[N_DENSE:]
    vperm[SP0:NFEAT_END, COLS - 1] = w_weight[0, N_DENSE:]
    vperm[0, COLS - 1] = float(w_bias[0])
    # partition-major [128, chunk, 625] so the DMA reads 26KB/partition runs
    vperm16 = np.ascontiguousarray(
        vperm.astype(np.float16).reshape(NCHUNK, P, COLS).transpose(1, 0, 2))

    r = np.arange(RTOT)
    in_sparse = (r >= SP0) & (r < NFEAT_END)
    off = np.where(in_sparse, (r - SP0) % SPARSE_DIM, 0)
    ramp = off.reshape(NCHUNK, P).T.astype(np.float32)
    ramp = np.ascontiguousarray(ramp)
    s_of_r = np.where(in_sparse, (r - SP0) // SPARSE_DIM, -1)
    return vperm16, ramp, s_of_r, in_sparse


def _prep_core(x_core, s_of_r, in_sparse):
    """Per-core idxrep[RTOT, bc] fp16 and dense xdn[13, bc] fp16."""
    bc = x_core.shape[0]
    idxrep = np.full((RTOT, bc), -1, np.int8)
    cols = (N_DENSE + s_of_r[in_sparse]).astype(np.int64)
    idxrep[in_sparse] = x_core[:, cols].T.astype(np.int8)
    # [group, p, chunk, gw] so each group sub-DMA reads contiguous
    # multi-KB runs per partition
    ngroups = bc // (GB * P)
    gw = GB * P
    idxrep = np.ascontiguousarray(
        idxrep.reshape(NCHUNK, P, ngroups, gw).transpose(2, 1, 0, 3))
    xdn = np.zeros((SP0, bc), np.float16)
    xdn[0] = 1.0
    xdn[1:1 + N_DENSE] = x_core[:, :N_DENSE].T.astype(np.float16)
    return idxrep, xdn


def run(x, w_weight, w_bias, v, trace=False, trace_kwargs=None):
    x = np.asarray(x, np.float32)
    w_weight = np.asarray(w_weight, np.float32)
    w_bias = np.asarray(w_bias, np.float32)
    v = np.asarray(v, np.float32)
    assert x.shape == (B_FULL, 39), x.shape

    vperm16, ramp, s_of_r, in_sparse = _prep_shared(w_weight, w_bias, v)
    in_maps = []
    for i in range(N_CORES):
        xc = x[i * BC:(i + 1) * BC]
        idxrep, xdn = _prep_core(xc, s_of_r, in_sparse)
        in_maps.append({
            "idxrep": idxrep,
            "xdn": xdn,
            "vperm": vperm16,
            "ramp": ramp,
        })

    nc = _get_program(BC)
    res = run_bass_kernel_spmd(
        nc, in_maps, list(range(N_CORES)),
        trace=trace, **(trace_kwargs or {}),
    )
    y = np.concatenate(
        [res.results[i]["y"].T.reshape(-1, 1) for i in range(N_CORES)], axis=0
    )
    return y.astype(np.float32), res


def kernel(x, w_weight, w_bias, v):
    y, _ = run(x, w_weight, w_bias, v)
    return y

